# revision 60
# baseline (speedup 1.0000x reference)
"""Llama-3.2 attention block (T=2048, H=2048, 32 q heads / 8 kv heads, d=64)
as a Bass/Tile kernel on 8 Trainium2 NeuronCores.

Sharding: tensor-parallel over heads. Core c owns q heads 4c..4c+3 and kv
head c (the GQA group). Each core projects its QKV shard over the full
sequence, applies RoPE, runs causal attention for its 4 heads. After EACH
head finishes, a per-head AllToAll ([8 dst, 64, 256] bf16) fires so the
collectives pipeline behind the remaining heads' compute; core c ends up
with the full 2048-feature attention output for its 256 sequence rows, then
o_proj runs in two passes (k-chunks for heads 0-2 first, head 3's chunks
after its collective lands) and writes a [256, 2048] fp32 output slice.

Layouts on device (bf16 matmul inputs, fp32 accumulation):
  - hidden and weights are pre-transposed on host so the contraction dim
    (hidden) lands on SBUF partitions.
  - QKV is produced transposed: q/k/v as [feat, seq] tiles. RoPE is applied
    in this layout: out = x * cos + (P @ x) * sin, where P is the
    rotate-half permutation done on the tensor engine.
  - q_t[m] holds heads 2m/2m+1 on partitions 0:64/64:128. Score matmuls
    contract over K=128 against zero-padded k tiles: k_t (k at 0:64, zeros
    above) serves even heads, k2_t (zeros, then k at 64:128) serves odd
    heads — the dead half of q contracts against zeros. K=64 matmuls
    measure ~2.3x slower per moving column on TRN2, hence the padding.
  - scores are computed transposed (scoresT[k, q]) so softmax's exp runs on
    the scalar engine and P@V consumes probsT directly as the moving
    operand; the softmax denominator rides along as a ones-column appended
    to V. No max-subtraction is needed: |scores| <= ~20 for this problem,
    safely inside fp32 exp range. 1/denom is broadcast across partitions
    with a K=1 ones matmul on the tensor engine.
  - cc-dependent loads (o_proj's lo chunks) ride the GPSIMD software-DGE
    queue, which is already serialized with the collectives — putting them
    on the sync or vector queues head-of-line blocks the staging/probs
    pipelines for the collective's full skew-absorption latency.
"""

import os
import sys
import types

import numpy as np
import ml_dtypes

T = 2048
HID = 2048
NH = 32
NKV = 8
D = 64
NCORES = 8
HPC = NH // NCORES        # q heads per core = 4
FPC = HPC * D             # attention feats per core = 256
SPC = T // NCORES         # seq chunk per core after AllToAll = 256
QKV_F = FPC + 2 * D       # per-core qkv proj feats = 384
ROPE_THETA = 500000.0
SCALE = float(D) ** -0.5

_CACHE = {}


def _ensure_trace_hooks():
    """Register the NTFF profiling hook that the stub antenv package lacks."""
    if "antenv.axon_hooks" in sys.modules:
        return
    try:
        import antenv
    except ImportError:
        return
    hooks = types.ModuleType("antenv.axon_hooks")
    holder = [None]
    hooks.set_axon_ntff_profile_hook = lambda h: holder.__setitem__(0, h)
    hooks.get_axon_ntff_profile_hook = lambda: holder[0]
    antenv.axon_hooks = hooks
    sys.modules["antenv.axon_hooks"] = hooks
    try:
        from trn_agent_boot.trn_boot import _ntff_profile_via_ctypes

        hook = _ntff_profile_via_ctypes("/opt/axon/libaxon_pjrt.so")
        if hook is not None:
            hooks.set_axon_ntff_profile_hook(hook)
    except Exception:
        pass


def _build():
    from contextlib import ExitStack

    from concourse import bacc
    import concourse.mybir as mybir
    import concourse.tile as tile
    from concourse.bass import ts
    from concourse.tile import add_dep_helper

    f32 = mybir.dt.float32
    bf16 = mybir.dt.bfloat16
    AF = mybir.ActivationFunctionType
    OP = mybir.AluOpType

    KO = HID // 128           # 16 contraction chunks
    NQ = T // 512             # 4 seq chunks of 512
    NB = T // 128             # 16 k blocks of 128

    nc = bacc.Bacc("TRN2", target_bir_lowering=False, debug=False, num_devices=NCORES)

    hT = nc.dram_tensor("hT", [T // 512, 128, HID // 128, 512], bf16, kind="ExternalInput")
    wT = nc.dram_tensor("wT", [HID, QKV_F], bf16, kind="ExternalInput")
    cosf = nc.dram_tensor("cosf", [128, T], bf16, kind="ExternalInput")
    sinf = nc.dram_tensor("sinf", [128, T], bf16, kind="ExternalInput")
    perm = nc.dram_tensor("perm", [128, 128], bf16, kind="ExternalInput")
    ident = nc.dram_tensor("ident", [128, 128], bf16, kind="ExternalInput")
    tri = nc.dram_tensor("tri", [128, 128], bf16, kind="ExternalInput")
    ones = nc.dram_tensor("ones", [128, 128], bf16, kind="ExternalInput")
    woT = nc.dram_tensor("woT", [HID, HID], bf16, kind="ExternalInput")
    out = nc.dram_tensor("out", [SPC, HID], f32, kind="ExternalOutput")
    # one AllToAll per head: [dst core, head feats, seq chunk]
    a2a_in = [
        nc.dram_tensor(f"a2a_in{g}", [NCORES, D, SPC], bf16) for g in range(HPC)
    ]
    a2a_out = [
        nc.dram_tensor(f"a2a_out{g}", [NCORES, D, SPC], bf16) for g in range(HPC)
    ]

    with tile.TileContext(nc) as tc, ExitStack() as ctx:
        consts = ctx.enter_context(tc.tile_pool(name="consts", bufs=1))
        persist = ctx.enter_context(tc.tile_pool(name="persist", bufs=1))

        # first QKV matmuls are gated on wt + the first hT chunk; load those
        # first, in k order, so compute starts as early as possible
        wt_t = consts.tile([128, KO, QKV_F], bf16, tag="wt")
        ht0_t = consts.tile([128, KO, 512], bf16, tag="ht0")
        nc.sync.dma_start(wt_t[:, 0, :], wT.ap()[ts(0, 128), :])
        nc.sync.dma_start(ht0_t[:, 0:1, :], hT.ap()[0, :, 0:1, :])
        for k4 in range(4):
            for k in range(4 * k4, 4 * k4 + 4):
                if k == 0:
                    continue
                nc.sync.dma_start(wt_t[:, k, :], wT.ap()[ts(k, 128), :])
            lo_k = 1 if k4 == 0 else 4 * k4
            nc.sync.dma_start(
                ht0_t[:, lo_k:4 * k4 + 4, :], hT.ap()[0, :, lo_k:4 * k4 + 4, :]
            )
        cos_t = consts.tile([128, T], bf16, tag="cos")
        nc.sync.dma_start(cos_t, cosf.ap())
        sin_t = consts.tile([128, T], bf16, tag="sin")
        nc.sync.dma_start(sin_t, sinf.ap())
        perm_t = consts.tile([128, 128], bf16, tag="perm")
        nc.sync.dma_start(perm_t, perm.ap())
        ident_t = consts.tile([128, 128], bf16, tag="ident")
        nc.sync.dma_start(ident_t, ident.ap())
        tri_t = consts.tile([128, 128], bf16, tag="tri")
        nc.sync.dma_start(tri_t, tri.ap())
        ones_t = consts.tile([128, 128], bf16, tag="ones")
        nc.sync.dma_start(ones_t, ones.ap())

        # Persistent activation tiles (live across phases A/B). q_t[m] holds
        # heads 2m (partitions 0:64) and 2m+1 (64:128); scores for the even
        # head contract against k_t (k at 0:64, zeros at 64:128), the odd head
        # against k2_t (zeros at 0:64, k at 64:128) — no per-head q copies.
        q_t = [persist.tile([128, T], bf16, tag=f"q{p}", name=f"q{p}") for p in range(2)]
        k_t = persist.tile([128, T], bf16, tag="kt")
        k2_t = persist.tile([128, T], bf16, tag="k2t")
        vlo_t = persist.tile([64, T], bf16, tag="vlo")
        vaug_t = persist.tile([128, KO, D + 1], bf16, tag="vaug")

        # zero the K-padding rows once (on gpsimd: idle at startup, and the
        # vector engine is on the QKV critical path)
        nc.gpsimd.memset(k_t[64:128, :], 0.0)
        nc.gpsimd.memset(k2_t[0:64, :], 0.0)

        # ---- Phase A: QKV projection + RoPE (outputs transposed [feat, seq]) ----
        with nc.named_scope("qkv"):
            with (
                tc.tile_pool(name="htp", bufs=2) as ht_pool,
                tc.tile_pool(name="atmp", bufs=3) as atmp,
                tc.tile_pool(name="psA", bufs=4, space="PSUM") as psA,
                tc.tile_pool(name="psAsh", bufs=2, space="PSUM") as psAsh,
                tc.tile_pool(name="psV", bufs=2, space="PSUM") as psV,
            ):
                nc.gpsimd.memset(vaug_t[:, :, D:D + 1], 1.0)
                for n in range(NQ):
                    if n == 0:
                        ht_t = ht0_t
                    else:
                        ht_t = ht_pool.tile([128, KO, 512], bf16, tag="ht")
                        for k4 in range(4):
                            nc.sync.dma_start(
                                ht_t[:, ts(k4, 4), :], hT.ap()[n, :, ts(k4, 4), :]
                            )
                    pending = None
                    for m in range(3):
                        pq = psA.tile([128, 512], f32, tag="pq")
                        for k in range(KO):
                            nc.tensor.matmul(
                                pq,
                                wt_t[:, k, ts(m, 128)],
                                ht_t[:, k, :],
                                start=(k == 0),
                                stop=(k == KO - 1),
                            )
                        # PSUM->SBUF cast on the scalar engine: it is idle all
                        # through QKV, while the vector queue carries the rope
                        # chain — keeps psA recycling off the vector backlog
                        xb = atmp.tile([128, 512], bf16, tag="xb")
                        nc.scalar.copy(xb, pq)
                        if pending is not None:
                            pending()
                            pending = None
                        if m < 2:
                            def rope_q(m=m, n=n, xb=xb):
                                # two q heads: rotate-half via PE perm + DVE
                                psh = psAsh.tile([128, 512], f32, tag="psh", name="psh")
                                nc.tensor.matmul(psh, perm_t, xb, start=True, stop=True)
                                t1 = atmp.tile([128, 512], f32, tag="t1", name="t1")
                                nc.vector.tensor_tensor(t1, xb, cos_t[:, ts(n, 512)], OP.mult)
                                t2 = atmp.tile([128, 512], f32, tag="t2", name="t2")
                                nc.vector.tensor_tensor(t2, psh, sin_t[:, ts(n, 512)], OP.mult)
                                nc.vector.tensor_tensor(q_t[m][:, ts(n, 512)], t1, t2, OP.add)
                            pending = rope_q
                        else:
                            def rope_kv(n=n, xb=xb):
                                # k head on partitions 0:64 (rope), v on 64:128
                                psh = psAsh.tile([128, 512], f32, tag="psh", name="psh")
                                nc.tensor.matmul(
                                    psh[0:64, :], perm_t[0:64, 0:64], xb[0:64, :],
                                    start=True, stop=True,
                                )
                                t1 = atmp.tile([128, 512], f32, tag="t1", name="t1")
                                nc.vector.tensor_tensor(
                                    t1[0:64, :], xb[0:64, :], cos_t[0:64, ts(n, 512)], OP.mult
                                )
                                t2 = atmp.tile([128, 512], f32, tag="t2", name="t2")
                                nc.vector.tensor_tensor(
                                    t2[0:64, :], psh[0:64, :], sin_t[0:64, ts(n, 512)], OP.mult
                                )
                                nc.vector.tensor_tensor(
                                    k_t[0:64, ts(n, 512)], t1[0:64, :], t2[0:64, :], OP.add
                                )
                                # odd heads' k copy (partition remap via DMA)
                                nc.sync.dma_start(
                                    k2_t[64:128, ts(n, 512)], k_t[0:64, ts(n, 512)]
                                )
                                # v slice to partitions 0:64 via DMA (partition remap)
                                nc.sync.dma_start(vlo_t[:, ts(n, 512)], xb[64:128, :])
                                for j in range(4 * n, 4 * n + 4):
                                    pv = psV.tile([128, D], bf16, tag="pv", name="pv")
                                    nc.tensor.transpose(
                                        pv, vlo_t[:, ts(j, 128)], ident_t[0:64, 0:64]
                                    )
                                    nc.vector.tensor_copy(vaug_t[:, j, 0:D], pv)
                            pending = rope_kv
                    if pending is not None:
                        pending()



        wo_t = consts.tile([128, KO, HID], bf16, tag="wo")
        lo_t = persist.tile([128, KO, SPC], bf16, tag="lo", name="lo")
        # w_o streams in right after the QKV input traffic drains; it must be
        # ahead of the attention staging writes so the collectives' completion
        # counters never queue behind these 8MB of transfers
        for k in range(KO):
            nc.sync.dma_start(wo_t[:, k, :], woT.ap()[ts(k, 128), :])

        # ---- Phase B: causal attention, 4 heads, scoresT layout ----
        a2a_dmas = []
        ccs = []

        def _emit_cc(g):
            cc = nc.gpsimd.collective_compute(
                "AllToAll",
                OP.bypass,
                replica_groups=[list(range(NCORES))],
                ins=[a2a_in[g].ap()],
                outs=[a2a_out[g].ap()],
            )
            for gg, dd in a2a_dmas:
                if gg == g:
                    add_dep_helper(cc.ins, dd.ins, sync=True, reason="cc waits a2a stage-in")
            return cc

        def _emit_lo(g):
            # o_proj k-chunks stream in as soon as the collective lands. These
            # ride the GPSIMD software-DGE queue: it is already serialized
            # with the collectives, so waiting on cc_g there never blocks the
            # staging/probs pipelines (the sync + vector queues would suffer
            # head-of-line blocking instead)
            for j in range(4):
                dl = nc.gpsimd.dma_start(
                    lo_t[:, 4 * g + j, :],
                    a2a_out[g].ap()[2 * j:2 * j + 2, :, :],
                )
                add_dep_helper(dl.ins, ccs[g].ins, sync=True, reason="o_proj waits AllToAll")

        with nc.named_scope("attn"):
            with (
                tc.tile_pool(name="probs", bufs=2) as probs_pool,
                tc.tile_pool(name="btmp", bufs=4) as btmp,
                tc.tile_pool(name="psS", bufs=2, space="PSUM") as psS,
                tc.tile_pool(name="psO", bufs=3, space="PSUM") as psO,
                tc.tile_pool(name="psB", bufs=1, space="PSUM") as psB,
            ):
                for h in range(HPC):
                    for i in range(NQ):
                        nj = 4 * i + 4
                        pr = probs_pool.tile([128, NB, 512], bf16, tag="pr")
                        po = psO.tile([D + 1, 512], f32, tag="po")
                        # every adjacent block pair shares one psum tile + exp
                        # call (the exp over a diagonal pair spans from the
                        # earlier block's causal offset; the later block's
                        # columns below its own offset hold exp(stale psum) —
                        # bounded, finite, and never read by P@V). P@V is
                        # lagged two chunks behind the scores so the PE never
                        # stalls on the exp chain.
                        chunks = [[j, j + 1] for j in range(0, nj, 2)]

                        kh_t = k_t if h % 2 == 0 else k2_t
                        qm_t = q_t[h // 2]

                        def emit_scores(js, i=i, pr=pr, kh_t=kh_t, qm_t=qm_t):
                            offs = [max(0, jj - 4 * i) * 128 for jj in js]
                            pss = psS.tile([128, 2, 512], f32, tag="pss", name="pss")
                            for u in range(2):
                                nc.tensor.matmul(
                                    pss[:, u, offs[u]:512],
                                    kh_t[:, ts(js[u], 128)],
                                    qm_t[:, i * 512 + offs[u]:(i + 1) * 512],
                                    start=True, stop=True,
                                )
                            nc.scalar.activation(
                                pr[:, js[0]:js[0] + 2, offs[0]:512],
                                pss[:, :, offs[0]:512],
                                AF.Exp, scale=SCALE,
                            )
                            for u in range(2):
                                r = js[u] - 4 * i
                                if r >= 0:  # block overlapping the causal diagonal
                                    off = offs[u]
                                    nc.vector.tensor_tensor(
                                        pr[:, js[u], off:off + 128],
                                        pr[:, js[u], off:off + 128],
                                        tri_t, OP.mult,
                                    )

                        def emit_pv(js, i=i, pr=pr, po=po, nj=nj):
                            for jj in js:
                                off = max(0, jj - 4 * i) * 128
                                nc.tensor.matmul(
                                    po[:, off:512], vaug_t[:, jj, :], pr[:, jj, off:512],
                                    start=(jj == 0), stop=(jj == nj - 1),
                                )

                        LAG = 2
                        for ci, ch in enumerate(chunks):
                            emit_scores(ch)
                            if ci >= LAG:
                                emit_pv(chunks[ci - LAG])
                        for ci in range(max(0, len(chunks) - LAG), len(chunks)):
                            emit_pv(chunks[ci])
                        # normalize: oT[f, q] = po[f, q] / den[q]; den row broadcast
                        # across partitions via a K=1 ones matmul, then 1/x on DVE
                        dbc = btmp.tile([D + 1, 512], bf16, tag="dbc")
                        nc.vector.tensor_copy(dbc[D:D + 1, :], po[D:D + 1, :])
                        pb = psB.tile([D, 512], f32, tag="pb")
                        nc.tensor.matmul(
                            pb, ones_t[D:D + 1, 0:D], dbc[D:D + 1, :],
                            start=True, stop=True,
                        )
                        rbs = btmp.tile([D, 512], f32, tag="rbs")
                        nc.vector.reciprocal_approx_fast(out=rbs, in_=pb)
                        oth = btmp.tile([D, 512], bf16, tag="oth")
                        nc.vector.tensor_tensor(oth, po[0:D, :], rbs, OP.mult)
                        for half in range(2):
                            dd = nc.sync.dma_start(
                                a2a_in[h].ap()[2 * i + half, :, :],
                                oth[:, ts(half, 256)],
                            )
                            a2a_dmas.append((h, dd))
                    # head h's outputs are fully staged: fire its AllToAll now
                    # so it overlaps the next head's attention compute, then
                    # pull its o_proj chunks in on the gpsimd queue
                    ccs.append(_emit_cc(h))
                    _emit_lo(h)

        # ---- Phase D: o_proj for this core's 256 seq rows ----
        # k chunks 0..11 (heads 0-2) have their collectives done well before
        # attention drains; k 12..15 wait only on the last head's collective.
        # Run all 8 PSUM groups' first 12 chunks, then the last 4 per group.
        with nc.named_scope("oproj"):
            with (
                tc.tile_pool(name="dtmp", bufs=3) as dtmp,
                tc.tile_pool(name="psD", bufs=1, space="PSUM") as psD,
            ):
                groups = [(m, e4) for m in range(SPC // 128) for e4 in range(HID // 512)]
                psos = [
                    psD.tile([128, 512], f32, tag=f"pso{g}", name=f"pso{g}")
                    for g in range(len(groups))
                ]
                KA = 12
                for g, (m, e4) in enumerate(groups):
                    for k in range(KA):
                        nc.tensor.matmul(
                            psos[g],
                            lo_t[:, k, ts(m, 128)],
                            wo_t[:, k, ts(e4, 512)],
                            start=(k == 0),
                            stop=False,
                        )
                for g, (m, e4) in enumerate(groups):
                    for k in range(KA, KO):
                        nc.tensor.matmul(
                            psos[g],
                            lo_t[:, k, ts(m, 128)],
                            wo_t[:, k, ts(e4, 512)],
                            start=False,
                            stop=(k == KO - 1),
                        )
                    ob = dtmp.tile([128, 512], f32, tag="ob")
                    nc.vector.tensor_copy(ob, psos[g])
                    nc.sync.dma_start(out.ap()[ts(m, 128), ts(e4, 512)], ob)

    nc.compile()
    return nc


def _get_nc():
    if "nc" not in _CACHE:
        _CACHE["nc"] = _build()
    return _CACHE["nc"]


def _host_prep(hidden_states, positions, w_qkv, w_o):
    bf16 = ml_dtypes.bfloat16
    hTb = np.ascontiguousarray(hidden_states.astype(np.float32).T).astype(bf16)
    # pretile to [n, p, ko, s] so each 512-seq chunk is one contiguous DMA
    hTt = np.ascontiguousarray(
        hTb.reshape(HID // 128, 128, T // 512, 512).transpose(2, 1, 0, 3)
    )
    woTb = np.ascontiguousarray(w_o.astype(np.float32).T).astype(bf16)
    # o_proj contraction order matches the per-head AllToAll arrival order:
    # head h of every core, h = 0..3
    rows = np.concatenate(
        [
            (np.arange(NCORES)[:, None] * FPC + h * D + np.arange(D)[None, :]).reshape(-1)
            for h in range(HPC)
        ]
    )
    woTb = np.ascontiguousarray(woTb[rows])

    inv = 1.0 / (ROPE_THETA ** (np.arange(0, D, 2, dtype=np.float32) / D))  # [32]
    ang = positions.astype(np.float32)[:, None] * inv[None, :]              # [T, 32]
    cos = np.cos(ang).T  # [32, T]
    sin = np.sin(ang).T
    p = np.arange(128)
    fr = (p % D) % (D // 2)
    sgn = np.where((p % D) < (D // 2), -1.0, 1.0).astype(np.float32)
    cosf = np.ascontiguousarray(cos[fr]).astype(bf16)                 # [128, T]
    sinf = np.ascontiguousarray(sin[fr] * sgn[:, None]).astype(bf16)  # [128, T]

    partner = np.where((p % D) < (D // 2), p + D // 2, p - D // 2)
    perm = np.zeros((128, 128), dtype=np.float32)
    perm[p, partner] = 1.0
    ident = np.eye(128, dtype=np.float32)
    tri = (np.arange(128)[None, :] >= np.arange(128)[:, None]).astype(np.float32)
    ones_m = np.ones((128, 128), dtype=np.float32)

    q_size = NH * D
    kv_size = NKV * D
    in_maps = []
    for c in range(NCORES):
        wq = w_qkv[c * FPC:(c + 1) * FPC]
        wk = w_qkv[q_size + c * D:q_size + (c + 1) * D]
        wv = w_qkv[q_size + kv_size + c * D:q_size + kv_size + (c + 1) * D]
        wTc = np.ascontiguousarray(
            np.concatenate([wq, wk, wv], axis=0).astype(np.float32).T
        ).astype(bf16)
        in_maps.append(
            {
                "hT": hTt,
                "wT": wTc,
                "cosf": cosf,
                "sinf": sinf,
                "perm": perm.astype(bf16),
                "ident": ident.astype(bf16),
                "tri": tri.astype(bf16),
                "ones": ones_m.astype(bf16),
                "woT": woTb,
            }
        )
    return in_maps


def run(inputs, trace=False):
    """Run on 8 NeuronCores; returns (full_output, BassKernelResults)."""
    if trace:
        _ensure_trace_hooks()
    from concourse import bass_utils

    if trace:
        bass_utils.upload_artifacts = lambda tmpdir: tmpdir
    nc = _get_nc()
    in_maps = _host_prep(
        np.asarray(inputs["hidden_states"]),
        np.asarray(inputs["positions"]),
        np.asarray(inputs["w_qkv"]),
        np.asarray(inputs["w_o"]),
    )
    res = bass_utils.run_bass_kernel_spmd(
        nc, in_maps, core_ids=list(range(NCORES)), trace=trace
    )
    full = np.concatenate(
        [res.results[c]["out"] for c in range(NCORES)], axis=0
    ).astype(np.float32)
    return full, res


def kernel(**inputs) -> np.ndarray:
    trace = bool(os.environ.get("KERNEL_TRACE"))
    full, _ = run(inputs, trace=trace)
    return full



# revision 61
# speedup vs baseline: 1.0661x; 1.0661x over previous
"""Llama-3.2 attention block (T=2048, H=2048, 32 q heads / 8 kv heads, d=64)
as a Bass/Tile kernel on 8 Trainium2 NeuronCores.

Sharding: tensor-parallel over heads. Core c owns q heads 4c..4c+3 and kv
head c (the GQA group). Each core projects its QKV shard over the full
sequence, applies RoPE, runs causal attention for its 4 heads. After EACH
head finishes, a per-head AllToAll ([8 dst, 64, 256] bf16) fires so the
collectives pipeline behind the remaining heads' compute; core c ends up
with the full 2048-feature attention output for its 256 sequence rows, then
o_proj runs in two passes (k-chunks for heads 0-2 first, head 3's chunks
after its collective lands) and writes a [256, 2048] fp32 output slice.

Layouts on device (bf16 matmul inputs, fp32 accumulation):
  - hidden and weights are pre-transposed on host so the contraction dim
    (hidden) lands on SBUF partitions.
  - QKV is produced transposed: q/k/v as [feat, seq] tiles. RoPE is applied
    in this layout: out = x * cos + (P @ x) * sin, where P is the
    rotate-half permutation done on the tensor engine.
  - q_t[m] holds heads 2m/2m+1 on partitions 0:64/64:128. Score matmuls
    contract over K=128 against zero-padded k tiles: k_t (k at 0:64, zeros
    above) serves even heads, k2_t (zeros, then k at 64:128) serves odd
    heads — the dead half of q contracts against zeros. K=64 matmuls
    measure ~2.3x slower per moving column on TRN2, hence the padding.
  - scores are computed transposed (scoresT[k, q]) so softmax's exp runs on
    the scalar engine and P@V consumes probsT directly as the moving
    operand; the softmax denominator rides along as a ones-column appended
    to V. No max-subtraction is needed: |scores| <= ~20 for this problem,
    safely inside fp32 exp range. 1/denom is broadcast across partitions
    with a K=1 ones matmul on the tensor engine.
  - cc-dependent loads (o_proj's lo chunks) ride the GPSIMD software-DGE
    queue, which is already serialized with the collectives — putting them
    on the sync or vector queues head-of-line blocks the staging/probs
    pipelines for the collective's full skew-absorption latency.
"""

import os
import sys
import types

import numpy as np
import ml_dtypes

T = 2048
HID = 2048
NH = 32
NKV = 8
D = 64
NCORES = 8
HPC = NH // NCORES        # q heads per core = 4
FPC = HPC * D             # attention feats per core = 256
SPC = T // NCORES         # seq chunk per core after AllToAll = 256
QKV_F = FPC + 2 * D       # per-core qkv proj feats = 384
ROPE_THETA = 500000.0
SCALE = float(D) ** -0.5

_CACHE = {}


def _ensure_trace_hooks():
    """Register the NTFF profiling hook that the stub antenv package lacks."""
    if "antenv.axon_hooks" in sys.modules:
        return
    try:
        import antenv
    except ImportError:
        return
    hooks = types.ModuleType("antenv.axon_hooks")
    holder = [None]
    hooks.set_axon_ntff_profile_hook = lambda h: holder.__setitem__(0, h)
    hooks.get_axon_ntff_profile_hook = lambda: holder[0]
    antenv.axon_hooks = hooks
    sys.modules["antenv.axon_hooks"] = hooks
    try:
        from trn_agent_boot.trn_boot import _ntff_profile_via_ctypes

        hook = _ntff_profile_via_ctypes("/opt/axon/libaxon_pjrt.so")
        if hook is not None:
            hooks.set_axon_ntff_profile_hook(hook)
    except Exception:
        pass


def _build():
    from contextlib import ExitStack

    from concourse import bacc
    import concourse.mybir as mybir
    import concourse.tile as tile
    from concourse.bass import ts
    from concourse.tile import add_dep_helper

    f32 = mybir.dt.float32
    bf16 = mybir.dt.bfloat16
    AF = mybir.ActivationFunctionType
    OP = mybir.AluOpType

    KO = HID // 128           # 16 contraction chunks
    NQ = T // 512             # 4 seq chunks of 512
    NB = T // 128             # 16 k blocks of 128

    nc = bacc.Bacc("TRN2", target_bir_lowering=False, debug=False, num_devices=NCORES)

    hT = nc.dram_tensor("hT", [T // 512, 128, HID // 128, 512], bf16, kind="ExternalInput")
    wT = nc.dram_tensor("wT", [HID, QKV_F], bf16, kind="ExternalInput")
    cosf = nc.dram_tensor("cosf", [128, T], bf16, kind="ExternalInput")
    sinf = nc.dram_tensor("sinf", [128, T], bf16, kind="ExternalInput")
    perm = nc.dram_tensor("perm", [128, 128], bf16, kind="ExternalInput")
    ident = nc.dram_tensor("ident", [128, 128], bf16, kind="ExternalInput")
    tri = nc.dram_tensor("tri", [128, 128], bf16, kind="ExternalInput")
    ones = nc.dram_tensor("ones", [128, 128], bf16, kind="ExternalInput")
    woT = nc.dram_tensor("woT", [HID, HID], bf16, kind="ExternalInput")
    out = nc.dram_tensor("out", [SPC, HID], f32, kind="ExternalOutput")
    # one AllToAll per head: [dst core, head feats, seq chunk]
    a2a_in = [
        nc.dram_tensor(f"a2a_in{g}", [NCORES, D, SPC], bf16) for g in range(HPC)
    ]
    a2a_out = [
        nc.dram_tensor(f"a2a_out{g}", [NCORES, D, SPC], bf16) for g in range(HPC)
    ]

    with tile.TileContext(nc) as tc, ExitStack() as ctx:
        consts = ctx.enter_context(tc.tile_pool(name="consts", bufs=1))
        persist = ctx.enter_context(tc.tile_pool(name="persist", bufs=1))

        # first QKV matmuls are gated on wt + the first hT chunk; load those
        # first, in k order, so compute starts as early as possible
        wt_t = consts.tile([128, KO, QKV_F], bf16, tag="wt")
        ht0_t = consts.tile([128, KO, 512], bf16, tag="ht0")
        nc.sync.dma_start(wt_t[:, 0, :], wT.ap()[ts(0, 128), :])
        nc.sync.dma_start(ht0_t[:, 0:1, :], hT.ap()[0, :, 0:1, :])
        for k4 in range(4):
            for k in range(4 * k4, 4 * k4 + 4):
                if k == 0:
                    continue
                nc.sync.dma_start(wt_t[:, k, :], wT.ap()[ts(k, 128), :])
            lo_k = 1 if k4 == 0 else 4 * k4
            nc.sync.dma_start(
                ht0_t[:, lo_k:4 * k4 + 4, :], hT.ap()[0, :, lo_k:4 * k4 + 4, :]
            )
        cos_t = consts.tile([128, T], bf16, tag="cos")
        nc.sync.dma_start(cos_t, cosf.ap())
        sin_t = consts.tile([128, T], bf16, tag="sin")
        nc.sync.dma_start(sin_t, sinf.ap())
        perm_t = consts.tile([128, 128], bf16, tag="perm")
        nc.sync.dma_start(perm_t, perm.ap())
        ident_t = consts.tile([128, 128], bf16, tag="ident")
        nc.sync.dma_start(ident_t, ident.ap())
        tri_t = consts.tile([128, 128], bf16, tag="tri")
        nc.sync.dma_start(tri_t, tri.ap())
        ones_t = consts.tile([128, 128], bf16, tag="ones")
        nc.sync.dma_start(ones_t, ones.ap())

        # Persistent activation tiles (live across phases A/B). q_t[m] holds
        # heads 2m (partitions 0:64) and 2m+1 (64:128); scores for the even
        # head contract against k_t (k at 0:64, zeros at 64:128), the odd head
        # against k2_t (zeros at 0:64, k at 64:128) — no per-head q copies.
        q_t = [persist.tile([128, T], bf16, tag=f"q{p}", name=f"q{p}") for p in range(2)]
        k_t = persist.tile([128, T], bf16, tag="kt")
        k2_t = persist.tile([128, T], bf16, tag="k2t")
        vlo_t = persist.tile([64, T], bf16, tag="vlo")
        vaug_t = persist.tile([128, KO, D + 1], bf16, tag="vaug")

        # zero the K-padding rows once (on gpsimd: idle at startup, and the
        # vector engine is on the QKV critical path)
        nc.gpsimd.memset(k_t[64:128, :], 0.0)
        nc.gpsimd.memset(k2_t[0:64, :], 0.0)

        # ---- Phase A: QKV projection + RoPE (outputs transposed [feat, seq]) ----
        with nc.named_scope("qkv"):
            with (
                tc.tile_pool(name="htp", bufs=2) as ht_pool,
                tc.tile_pool(name="atmp", bufs=3) as atmp,
                tc.tile_pool(name="psA", bufs=4, space="PSUM") as psA,
                tc.tile_pool(name="psAsh", bufs=2, space="PSUM") as psAsh,
                tc.tile_pool(name="psV", bufs=2, space="PSUM") as psV,
            ):
                nc.gpsimd.memset(vaug_t[:, :, D:D + 1], 1.0)
                for n in range(NQ):
                    if n == 0:
                        ht_t = ht0_t
                    else:
                        ht_t = ht_pool.tile([128, KO, 512], bf16, tag="ht")
                        for k4 in range(4):
                            nc.sync.dma_start(
                                ht_t[:, ts(k4, 4), :], hT.ap()[n, :, ts(k4, 4), :]
                            )
                    pending = None
                    for m in range(3):
                        pq = psA.tile([128, 512], f32, tag="pq")
                        for k in range(KO):
                            nc.tensor.matmul(
                                pq,
                                wt_t[:, k, ts(m, 128)],
                                ht_t[:, k, :],
                                start=(k == 0),
                                stop=(k == KO - 1),
                            )
                        # PSUM->SBUF cast on the scalar engine: it is idle all
                        # through QKV, while the vector queue carries the rope
                        # chain — keeps psA recycling off the vector backlog
                        xb = atmp.tile([128, 512], bf16, tag="xb")
                        nc.scalar.copy(xb, pq)
                        if pending is not None:
                            pending()
                            pending = None
                        if m < 2:
                            def rope_q(m=m, n=n, xb=xb):
                                # two q heads: rotate-half via PE perm + DVE
                                psh = psAsh.tile([128, 512], f32, tag="psh", name="psh")
                                nc.tensor.matmul(psh, perm_t, xb, start=True, stop=True)
                                t1 = atmp.tile([128, 512], f32, tag="t1", name="t1")
                                nc.vector.tensor_tensor(t1, xb, cos_t[:, ts(n, 512)], OP.mult)
                                t2 = atmp.tile([128, 512], f32, tag="t2", name="t2")
                                nc.vector.tensor_tensor(t2, psh, sin_t[:, ts(n, 512)], OP.mult)
                                nc.vector.tensor_tensor(q_t[m][:, ts(n, 512)], t1, t2, OP.add)
                            pending = rope_q
                        else:
                            def rope_kv(n=n, xb=xb):
                                # k head on partitions 0:64 (rope), v on 64:128
                                psh = psAsh.tile([128, 512], f32, tag="psh", name="psh")
                                nc.tensor.matmul(
                                    psh[0:64, :], perm_t[0:64, 0:64], xb[0:64, :],
                                    start=True, stop=True,
                                )
                                t1 = atmp.tile([128, 512], f32, tag="t1", name="t1")
                                nc.vector.tensor_tensor(
                                    t1[0:64, :], xb[0:64, :], cos_t[0:64, ts(n, 512)], OP.mult
                                )
                                t2 = atmp.tile([128, 512], f32, tag="t2", name="t2")
                                nc.vector.tensor_tensor(
                                    t2[0:64, :], psh[0:64, :], sin_t[0:64, ts(n, 512)], OP.mult
                                )
                                nc.vector.tensor_tensor(
                                    k_t[0:64, ts(n, 512)], t1[0:64, :], t2[0:64, :], OP.add
                                )
                                # odd heads' k copy (partition remap via DMA)
                                nc.sync.dma_start(
                                    k2_t[64:128, ts(n, 512)], k_t[0:64, ts(n, 512)]
                                )
                                # v slice to partitions 0:64 via DMA (partition remap)
                                nc.sync.dma_start(vlo_t[:, ts(n, 512)], xb[64:128, :])
                                for j in range(4 * n, 4 * n + 4):
                                    pv = psV.tile([128, D], bf16, tag="pv", name="pv")
                                    nc.tensor.transpose(
                                        pv, vlo_t[:, ts(j, 128)], ident_t[0:64, 0:64]
                                    )
                                    nc.vector.tensor_copy(vaug_t[:, j, 0:D], pv)
                            pending = rope_kv
                    if pending is not None:
                        pending()



        wo_t = consts.tile([128, KO, HID], bf16, tag="wo")
        lo_t = persist.tile([128, KO, SPC], bf16, tag="lo", name="lo")
        # w_o streams in right after the QKV input traffic drains; it must be
        # ahead of the attention staging writes so the collectives' completion
        # counters never queue behind these 8MB of transfers
        for k in range(KO):
            nc.sync.dma_start(wo_t[:, k, :], woT.ap()[ts(k, 128), :])

        # ---- Phase B: causal attention, 4 heads, scoresT layout ----
        a2a_dmas = []
        ccs = []

        def _emit_cc(g):
            cc = nc.gpsimd.collective_compute(
                "AllToAll",
                OP.bypass,
                replica_groups=[list(range(NCORES))],
                ins=[a2a_in[g].ap()],
                outs=[a2a_out[g].ap()],
            )
            for gg, dd in a2a_dmas:
                if gg == g:
                    add_dep_helper(cc.ins, dd.ins, sync=True, reason="cc waits a2a stage-in")
            return cc

        def _emit_lo(g):
            # o_proj k-chunks stream in as soon as the collective lands. These
            # ride the GPSIMD software-DGE queue: it is already serialized
            # with the collectives, so waiting on cc_g there never blocks the
            # staging/probs pipelines (the sync + vector queues would suffer
            # head-of-line blocking instead)
            for j in range(4):
                dl = nc.gpsimd.dma_start(
                    lo_t[:, 4 * g + j, :],
                    a2a_out[g].ap()[2 * j:2 * j + 2, :, :],
                )
                add_dep_helper(dl.ins, ccs[g].ins, sync=True, reason="o_proj waits AllToAll")

        with nc.named_scope("attn"):
            with (
                tc.tile_pool(name="probs", bufs=2) as probs_pool,
                tc.tile_pool(name="btmp", bufs=4) as btmp,
                tc.tile_pool(name="psS", bufs=2, space="PSUM") as psS,
                tc.tile_pool(name="psO", bufs=2, space="PSUM") as psO,
                tc.tile_pool(name="psB", bufs=2, space="PSUM") as psB,
            ):
                for h in range(HPC):
                    for i in range(NQ):
                        nj = 4 * i + 4
                        pr = probs_pool.tile([128, NB, 512], bf16, tag="pr")
                        po = psO.tile([D + 1, 512], f32, tag="po")
                        # every adjacent block pair shares one psum tile + exp
                        # call (the exp over a diagonal pair spans from the
                        # earlier block's causal offset; the later block's
                        # columns below its own offset hold exp(stale psum) —
                        # bounded, finite, and never read by P@V). P@V is
                        # lagged two chunks behind the scores so the PE never
                        # stalls on the exp chain.
                        chunks = [[j, j + 1] for j in range(0, nj, 2)]

                        kh_t = k_t if h % 2 == 0 else k2_t
                        qm_t = q_t[h // 2]

                        def emit_scores(js, i=i, pr=pr, kh_t=kh_t, qm_t=qm_t):
                            offs = [max(0, jj - 4 * i) * 128 for jj in js]
                            pss = psS.tile([128, 2, 512], f32, tag="pss", name="pss")
                            for u in range(2):
                                nc.tensor.matmul(
                                    pss[:, u, offs[u]:512],
                                    kh_t[:, ts(js[u], 128)],
                                    qm_t[:, i * 512 + offs[u]:(i + 1) * 512],
                                    start=True, stop=True,
                                )
                            nc.scalar.activation(
                                pr[:, js[0]:js[0] + 2, offs[0]:512],
                                pss[:, :, offs[0]:512],
                                AF.Exp, scale=SCALE,
                            )
                            for u in range(2):
                                r = js[u] - 4 * i
                                if r >= 0:  # block overlapping the causal diagonal
                                    off = offs[u]
                                    nc.vector.tensor_tensor(
                                        pr[:, js[u], off:off + 128],
                                        pr[:, js[u], off:off + 128],
                                        tri_t, OP.mult,
                                    )

                        def emit_pv(js, i=i, pr=pr, po=po, nj=nj):
                            for jj in js:
                                off = max(0, jj - 4 * i) * 128
                                nc.tensor.matmul(
                                    po[:, off:512], vaug_t[:, jj, :], pr[:, jj, off:512],
                                    start=(jj == 0), stop=(jj == nj - 1),
                                )

                        LAG = 2
                        for ci, ch in enumerate(chunks):
                            emit_scores(ch)
                            if ci >= LAG:
                                emit_pv(chunks[ci - LAG])
                        for ci in range(max(0, len(chunks) - LAG), len(chunks)):
                            emit_pv(chunks[ci])
                        # normalize: oT[f, q] = po[f, q] / den[q]; den row broadcast
                        # across partitions via a K=1 ones matmul, then 1/x on DVE
                        dbc = btmp.tile([D + 1, 512], bf16, tag="dbc")
                        nc.vector.tensor_copy(dbc[D:D + 1, :], po[D:D + 1, :])
                        pb = psB.tile([D, 512], f32, tag="pb")
                        nc.tensor.matmul(
                            pb, ones_t[D:D + 1, 0:D], dbc[D:D + 1, :],
                            start=True, stop=True,
                        )
                        rbs = btmp.tile([D, 512], f32, tag="rbs")
                        nc.vector.reciprocal_approx_fast(out=rbs, in_=pb)
                        oth = btmp.tile([D, 512], bf16, tag="oth")
                        nc.vector.tensor_tensor(oth, po[0:D, :], rbs, OP.mult)
                        for half in range(2):
                            dd = nc.sync.dma_start(
                                a2a_in[h].ap()[2 * i + half, :, :],
                                oth[:, ts(half, 256)],
                            )
                            a2a_dmas.append((h, dd))
                    # head h's outputs are fully staged: fire its AllToAll now
                    # so it overlaps the next head's attention compute, then
                    # pull its o_proj chunks in on the gpsimd queue
                    ccs.append(_emit_cc(h))
                    _emit_lo(h)

        # ---- Phase D: o_proj for this core's 256 seq rows ----
        # k chunks 0..11 (heads 0-2) have their collectives done well before
        # attention drains; k 12..15 wait only on the last head's collective.
        # Run all 8 PSUM groups' first 12 chunks, then the last 4 per group.
        with nc.named_scope("oproj"):
            with (
                tc.tile_pool(name="dtmp", bufs=3) as dtmp,
                tc.tile_pool(name="psD", bufs=1, space="PSUM") as psD,
            ):
                groups = [(m, e4) for m in range(SPC // 128) for e4 in range(HID // 512)]
                psos = [
                    psD.tile([128, 512], f32, tag=f"pso{g}", name=f"pso{g}")
                    for g in range(len(groups))
                ]
                KA = 12
                for g, (m, e4) in enumerate(groups):
                    for k in range(KA):
                        nc.tensor.matmul(
                            psos[g],
                            lo_t[:, k, ts(m, 128)],
                            wo_t[:, k, ts(e4, 512)],
                            start=(k == 0),
                            stop=False,
                        )
                for g, (m, e4) in enumerate(groups):
                    for k in range(KA, KO):
                        nc.tensor.matmul(
                            psos[g],
                            lo_t[:, k, ts(m, 128)],
                            wo_t[:, k, ts(e4, 512)],
                            start=False,
                            stop=(k == KO - 1),
                        )
                    ob = dtmp.tile([128, 512], f32, tag="ob")
                    nc.vector.tensor_copy(ob, psos[g])
                    nc.sync.dma_start(out.ap()[ts(m, 128), ts(e4, 512)], ob)

    nc.compile()
    return nc


def _get_nc():
    if "nc" not in _CACHE:
        _CACHE["nc"] = _build()
    return _CACHE["nc"]


def _host_prep(hidden_states, positions, w_qkv, w_o):
    bf16 = ml_dtypes.bfloat16
    hTb = np.ascontiguousarray(hidden_states.astype(np.float32).T).astype(bf16)
    # pretile to [n, p, ko, s] so each 512-seq chunk is one contiguous DMA
    hTt = np.ascontiguousarray(
        hTb.reshape(HID // 128, 128, T // 512, 512).transpose(2, 1, 0, 3)
    )
    woTb = np.ascontiguousarray(w_o.astype(np.float32).T).astype(bf16)
    # o_proj contraction order matches the per-head AllToAll arrival order:
    # head h of every core, h = 0..3
    rows = np.concatenate(
        [
            (np.arange(NCORES)[:, None] * FPC + h * D + np.arange(D)[None, :]).reshape(-1)
            for h in range(HPC)
        ]
    )
    woTb = np.ascontiguousarray(woTb[rows])

    inv = 1.0 / (ROPE_THETA ** (np.arange(0, D, 2, dtype=np.float32) / D))  # [32]
    ang = positions.astype(np.float32)[:, None] * inv[None, :]              # [T, 32]
    cos = np.cos(ang).T  # [32, T]
    sin = np.sin(ang).T
    p = np.arange(128)
    fr = (p % D) % (D // 2)
    sgn = np.where((p % D) < (D // 2), -1.0, 1.0).astype(np.float32)
    cosf = np.ascontiguousarray(cos[fr]).astype(bf16)                 # [128, T]
    sinf = np.ascontiguousarray(sin[fr] * sgn[:, None]).astype(bf16)  # [128, T]

    partner = np.where((p % D) < (D // 2), p + D // 2, p - D // 2)
    perm = np.zeros((128, 128), dtype=np.float32)
    perm[p, partner] = 1.0
    ident = np.eye(128, dtype=np.float32)
    tri = (np.arange(128)[None, :] >= np.arange(128)[:, None]).astype(np.float32)
    ones_m = np.ones((128, 128), dtype=np.float32)

    q_size = NH * D
    kv_size = NKV * D
    in_maps = []
    for c in range(NCORES):
        wq = w_qkv[c * FPC:(c + 1) * FPC]
        wk = w_qkv[q_size + c * D:q_size + (c + 1) * D]
        wv = w_qkv[q_size + kv_size + c * D:q_size + kv_size + (c + 1) * D]
        wTc = np.ascontiguousarray(
            np.concatenate([wq, wk, wv], axis=0).astype(np.float32).T
        ).astype(bf16)
        in_maps.append(
            {
                "hT": hTt,
                "wT": wTc,
                "cosf": cosf,
                "sinf": sinf,
                "perm": perm.astype(bf16),
                "ident": ident.astype(bf16),
                "tri": tri.astype(bf16),
                "ones": ones_m.astype(bf16),
                "woT": woTb,
            }
        )
    return in_maps


def run(inputs, trace=False):
    """Run on 8 NeuronCores; returns (full_output, BassKernelResults)."""
    if trace:
        _ensure_trace_hooks()
    from concourse import bass_utils

    if trace:
        bass_utils.upload_artifacts = lambda tmpdir: tmpdir
    nc = _get_nc()
    in_maps = _host_prep(
        np.asarray(inputs["hidden_states"]),
        np.asarray(inputs["positions"]),
        np.asarray(inputs["w_qkv"]),
        np.asarray(inputs["w_o"]),
    )
    res = bass_utils.run_bass_kernel_spmd(
        nc, in_maps, core_ids=list(range(NCORES)), trace=trace
    )
    full = np.concatenate(
        [res.results[c]["out"] for c in range(NCORES)], axis=0
    ).astype(np.float32)
    return full, res


def kernel(**inputs) -> np.ndarray:
    trace = bool(os.environ.get("KERNEL_TRACE"))
    full, _ = run(inputs, trace=trace)
    return full



# revision 62
# speedup vs baseline: 1.0726x; 1.0061x over previous
"""Llama-3.2 attention block (T=2048, H=2048, 32 q heads / 8 kv heads, d=64)
as a Bass/Tile kernel on 8 Trainium2 NeuronCores.

Sharding: tensor-parallel over heads. Core c owns q heads 4c..4c+3 and kv
head c (the GQA group). Each core projects its QKV shard over the full
sequence, applies RoPE, runs causal attention for its 4 heads. After EACH
head finishes, a per-head AllToAll ([8 dst, 64, 256] bf16) fires so the
collectives pipeline behind the remaining heads' compute; core c ends up
with the full 2048-feature attention output for its 256 sequence rows, then
o_proj runs in two passes (k-chunks for heads 0-2 first, head 3's chunks
after its collective lands) and writes a [256, 2048] fp32 output slice.

Layouts on device (bf16 matmul inputs, fp32 accumulation):
  - hidden and weights are pre-transposed on host so the contraction dim
    (hidden) lands on SBUF partitions.
  - QKV is produced transposed: q/k/v as [feat, seq] tiles. RoPE is applied
    in this layout: out = x * cos + (P @ x) * sin, where P is the
    rotate-half permutation done on the tensor engine.
  - q_t[m] holds heads 2m/2m+1 on partitions 0:64/64:128. Score matmuls
    contract over K=128 against zero-padded k tiles: k_t (k at 0:64, zeros
    above) serves even heads, k2_t (zeros, then k at 64:128) serves odd
    heads — the dead half of q contracts against zeros. K=64 matmuls
    measure ~2.3x slower per moving column on TRN2, hence the padding.
  - scores are computed transposed (scoresT[k, q]) so softmax's exp runs on
    the scalar engine and P@V consumes probsT directly as the moving
    operand; the softmax denominator rides along as a ones-column appended
    to V. No max-subtraction is needed: |scores| <= ~20 for this problem,
    safely inside fp32 exp range. 1/denom is broadcast across partitions
    with a K=1 ones matmul on the tensor engine.
  - cc-dependent loads (o_proj's lo chunks) ride the GPSIMD software-DGE
    queue, which is already serialized with the collectives — putting them
    on the sync or vector queues head-of-line blocks the staging/probs
    pipelines for the collective's full skew-absorption latency.
"""

import os
import sys
import types

import numpy as np
import ml_dtypes

T = 2048
HID = 2048
NH = 32
NKV = 8
D = 64
NCORES = 8
HPC = NH // NCORES        # q heads per core = 4
FPC = HPC * D             # attention feats per core = 256
SPC = T // NCORES         # seq chunk per core after AllToAll = 256
QKV_F = FPC + 2 * D       # per-core qkv proj feats = 384
ROPE_THETA = 500000.0
SCALE = float(D) ** -0.5

_CACHE = {}


def _ensure_trace_hooks():
    """Register the NTFF profiling hook that the stub antenv package lacks."""
    if "antenv.axon_hooks" in sys.modules:
        return
    try:
        import antenv
    except ImportError:
        return
    hooks = types.ModuleType("antenv.axon_hooks")
    holder = [None]
    hooks.set_axon_ntff_profile_hook = lambda h: holder.__setitem__(0, h)
    hooks.get_axon_ntff_profile_hook = lambda: holder[0]
    antenv.axon_hooks = hooks
    sys.modules["antenv.axon_hooks"] = hooks
    try:
        from trn_agent_boot.trn_boot import _ntff_profile_via_ctypes

        hook = _ntff_profile_via_ctypes("/opt/axon/libaxon_pjrt.so")
        if hook is not None:
            hooks.set_axon_ntff_profile_hook(hook)
    except Exception:
        pass


def _build():
    from contextlib import ExitStack

    from concourse import bacc
    import concourse.mybir as mybir
    import concourse.tile as tile
    from concourse.bass import ts
    from concourse.tile import add_dep_helper

    f32 = mybir.dt.float32
    bf16 = mybir.dt.bfloat16
    AF = mybir.ActivationFunctionType
    OP = mybir.AluOpType

    KO = HID // 128           # 16 contraction chunks
    NQ = T // 512             # 4 seq chunks of 512
    NB = T // 128             # 16 k blocks of 128

    nc = bacc.Bacc("TRN2", target_bir_lowering=False, debug=False, num_devices=NCORES)

    hT = nc.dram_tensor("hT", [T // 512, 128, HID // 128, 512], bf16, kind="ExternalInput")
    wT = nc.dram_tensor("wT", [HID, QKV_F], bf16, kind="ExternalInput")
    cosf = nc.dram_tensor("cosf", [128, T], bf16, kind="ExternalInput")
    sinf = nc.dram_tensor("sinf", [128, T], bf16, kind="ExternalInput")
    perm = nc.dram_tensor("perm", [128, 128], bf16, kind="ExternalInput")
    ident = nc.dram_tensor("ident", [128, 128], bf16, kind="ExternalInput")
    tri = nc.dram_tensor("tri", [128, 128], bf16, kind="ExternalInput")
    ones = nc.dram_tensor("ones", [128, 128], bf16, kind="ExternalInput")
    woT = nc.dram_tensor("woT", [HID, HID], bf16, kind="ExternalInput")
    out = nc.dram_tensor("out", [SPC, HID], f32, kind="ExternalOutput")
    # one AllToAll per head: [dst core, head feats, seq chunk]
    a2a_in = [
        nc.dram_tensor(f"a2a_in{g}", [NCORES, D, SPC], bf16) for g in range(HPC)
    ]
    a2a_out = [
        nc.dram_tensor(f"a2a_out{g}", [NCORES, D, SPC], bf16) for g in range(HPC)
    ]

    with tile.TileContext(nc) as tc, ExitStack() as ctx:
        consts = ctx.enter_context(tc.tile_pool(name="consts", bufs=1))
        persist = ctx.enter_context(tc.tile_pool(name="persist", bufs=1))

        # first QKV matmuls are gated on wt + the first hT chunk; load those
        # first, in k order, so compute starts as early as possible
        wt_t = consts.tile([128, KO, QKV_F], bf16, tag="wt")
        ht0_t = consts.tile([128, KO, 512], bf16, tag="ht0")
        # k-interleaved at single-chunk granularity for the first group so the
        # accumulation never outruns the data, coarser batches after
        for k in range(4):
            nc.sync.dma_start(wt_t[:, k, :], wT.ap()[ts(k, 128), :])
            nc.sync.dma_start(ht0_t[:, k:k + 1, :], hT.ap()[0, :, k:k + 1, :])
        for k4 in range(1, 4):
            for k in range(4 * k4, 4 * k4 + 4):
                nc.sync.dma_start(wt_t[:, k, :], wT.ap()[ts(k, 128), :])
            nc.sync.dma_start(
                ht0_t[:, ts(k4, 4), :], hT.ap()[0, :, ts(k4, 4), :]
            )
        cos_t = consts.tile([128, T], bf16, tag="cos")
        nc.sync.dma_start(cos_t, cosf.ap())
        sin_t = consts.tile([128, T], bf16, tag="sin")
        nc.sync.dma_start(sin_t, sinf.ap())
        perm_t = consts.tile([128, 128], bf16, tag="perm")
        nc.sync.dma_start(perm_t, perm.ap())
        ident_t = consts.tile([128, 128], bf16, tag="ident")
        nc.sync.dma_start(ident_t, ident.ap())
        tri_t = consts.tile([128, 128], bf16, tag="tri")
        nc.sync.dma_start(tri_t, tri.ap())
        ones_t = consts.tile([128, 128], bf16, tag="ones")
        nc.sync.dma_start(ones_t, ones.ap())

        # Persistent activation tiles (live across phases A/B). q_t[m] holds
        # heads 2m (partitions 0:64) and 2m+1 (64:128); scores for the even
        # head contract against k_t (k at 0:64, zeros at 64:128), the odd head
        # against k2_t (zeros at 0:64, k at 64:128) — no per-head q copies.
        q_t = [persist.tile([128, T], bf16, tag=f"q{p}", name=f"q{p}") for p in range(2)]
        k_t = persist.tile([128, T], bf16, tag="kt")
        k2_t = persist.tile([128, T], bf16, tag="k2t")
        vlo_t = persist.tile([64, T], bf16, tag="vlo")
        vaug_t = persist.tile([128, KO, D + 1], bf16, tag="vaug")

        # zero the K-padding rows once (on gpsimd: idle at startup, and the
        # vector engine is on the QKV critical path)
        nc.gpsimd.memset(k_t[64:128, :], 0.0)
        nc.gpsimd.memset(k2_t[0:64, :], 0.0)

        # ---- Phase A: QKV projection + RoPE (outputs transposed [feat, seq]) ----
        with nc.named_scope("qkv"):
            with (
                tc.tile_pool(name="htp", bufs=2) as ht_pool,
                tc.tile_pool(name="atmp", bufs=3) as atmp,
                tc.tile_pool(name="psA", bufs=4, space="PSUM") as psA,
                tc.tile_pool(name="psAsh", bufs=2, space="PSUM") as psAsh,
                tc.tile_pool(name="psV", bufs=2, space="PSUM") as psV,
            ):
                nc.gpsimd.memset(vaug_t[:, :, D:D + 1], 1.0)
                for n in range(NQ):
                    if n == 0:
                        ht_t = ht0_t
                    else:
                        ht_t = ht_pool.tile([128, KO, 512], bf16, tag="ht")
                        for k4 in range(4):
                            nc.sync.dma_start(
                                ht_t[:, ts(k4, 4), :], hT.ap()[n, :, ts(k4, 4), :]
                            )
                    pending = None
                    for m in range(3):
                        pq = psA.tile([128, 512], f32, tag="pq")
                        for k in range(KO):
                            nc.tensor.matmul(
                                pq,
                                wt_t[:, k, ts(m, 128)],
                                ht_t[:, k, :],
                                start=(k == 0),
                                stop=(k == KO - 1),
                            )
                        # PSUM->SBUF cast on the scalar engine: it is idle all
                        # through QKV, while the vector queue carries the rope
                        # chain — keeps psA recycling off the vector backlog
                        xb = atmp.tile([128, 512], bf16, tag="xb")
                        nc.scalar.copy(xb, pq)
                        if pending is not None:
                            pending()
                            pending = None
                        if m < 2:
                            def rope_q(m=m, n=n, xb=xb):
                                # two q heads: rotate-half via PE perm + DVE
                                psh = psAsh.tile([128, 512], f32, tag="psh", name="psh")
                                nc.tensor.matmul(psh, perm_t, xb, start=True, stop=True)
                                t1 = atmp.tile([128, 512], f32, tag="t1", name="t1")
                                nc.vector.tensor_tensor(t1, xb, cos_t[:, ts(n, 512)], OP.mult)
                                t2 = atmp.tile([128, 512], f32, tag="t2", name="t2")
                                nc.vector.tensor_tensor(t2, psh, sin_t[:, ts(n, 512)], OP.mult)
                                nc.vector.tensor_tensor(q_t[m][:, ts(n, 512)], t1, t2, OP.add)
                            pending = rope_q
                        else:
                            def rope_kv(n=n, xb=xb):
                                # k head on partitions 0:64 (rope), v on 64:128
                                psh = psAsh.tile([128, 512], f32, tag="psh", name="psh")
                                nc.tensor.matmul(
                                    psh[0:64, :], perm_t[0:64, 0:64], xb[0:64, :],
                                    start=True, stop=True,
                                )
                                t1 = atmp.tile([128, 512], f32, tag="t1", name="t1")
                                nc.vector.tensor_tensor(
                                    t1[0:64, :], xb[0:64, :], cos_t[0:64, ts(n, 512)], OP.mult
                                )
                                t2 = atmp.tile([128, 512], f32, tag="t2", name="t2")
                                nc.vector.tensor_tensor(
                                    t2[0:64, :], psh[0:64, :], sin_t[0:64, ts(n, 512)], OP.mult
                                )
                                nc.vector.tensor_tensor(
                                    k_t[0:64, ts(n, 512)], t1[0:64, :], t2[0:64, :], OP.add
                                )
                                # odd heads' k copy (partition remap via DMA)
                                nc.sync.dma_start(
                                    k2_t[64:128, ts(n, 512)], k_t[0:64, ts(n, 512)]
                                )
                                # v slice to partitions 0:64 via DMA (partition remap)
                                nc.sync.dma_start(vlo_t[:, ts(n, 512)], xb[64:128, :])
                                for j in range(4 * n, 4 * n + 4):
                                    pv = psV.tile([128, D], bf16, tag="pv", name="pv")
                                    nc.tensor.transpose(
                                        pv, vlo_t[:, ts(j, 128)], ident_t[0:64, 0:64]
                                    )
                                    nc.vector.tensor_copy(vaug_t[:, j, 0:D], pv)
                            pending = rope_kv
                    if pending is not None:
                        pending()



        wo_t = consts.tile([128, KO, HID], bf16, tag="wo")
        lo_t = persist.tile([128, KO, SPC], bf16, tag="lo", name="lo")
        # w_o streams in right after the QKV input traffic drains; it must be
        # ahead of the attention staging writes so the collectives' completion
        # counters never queue behind these 8MB of transfers
        for k in range(KO):
            nc.sync.dma_start(wo_t[:, k, :], woT.ap()[ts(k, 128), :])

        # ---- Phase B: causal attention, 4 heads, scoresT layout ----
        a2a_dmas = []
        ccs = []

        def _emit_cc(g):
            cc = nc.gpsimd.collective_compute(
                "AllToAll",
                OP.bypass,
                replica_groups=[list(range(NCORES))],
                ins=[a2a_in[g].ap()],
                outs=[a2a_out[g].ap()],
            )
            for gg, dd in a2a_dmas:
                if gg == g:
                    add_dep_helper(cc.ins, dd.ins, sync=True, reason="cc waits a2a stage-in")
            return cc

        def _emit_lo(g):
            # o_proj k-chunks stream in as soon as the collective lands. These
            # ride the GPSIMD software-DGE queue: it is already serialized
            # with the collectives, so waiting on cc_g there never blocks the
            # staging/probs pipelines (the sync + vector queues would suffer
            # head-of-line blocking instead)
            for j in range(4):
                dl = nc.gpsimd.dma_start(
                    lo_t[:, 4 * g + j, :],
                    a2a_out[g].ap()[2 * j:2 * j + 2, :, :],
                )
                add_dep_helper(dl.ins, ccs[g].ins, sync=True, reason="o_proj waits AllToAll")

        with nc.named_scope("attn"):
            with (
                tc.tile_pool(name="probs", bufs=2) as probs_pool,
                tc.tile_pool(name="btmp", bufs=4) as btmp,
                tc.tile_pool(name="psS", bufs=2, space="PSUM") as psS,
                tc.tile_pool(name="psO", bufs=2, space="PSUM") as psO,
                tc.tile_pool(name="psB", bufs=2, space="PSUM") as psB,
            ):
                for h in range(HPC):
                    for i in range(NQ):
                        nj = 4 * i + 4
                        pr = probs_pool.tile([128, NB, 512], bf16, tag="pr")
                        po = psO.tile([D + 1, 512], f32, tag="po")
                        # every adjacent block pair shares one psum tile + exp
                        # call (the exp over a diagonal pair spans from the
                        # earlier block's causal offset; the later block's
                        # columns below its own offset hold exp(stale psum) —
                        # bounded, finite, and never read by P@V). P@V is
                        # lagged two chunks behind the scores so the PE never
                        # stalls on the exp chain.
                        chunks = [[j, j + 1] for j in range(0, nj, 2)]

                        kh_t = k_t if h % 2 == 0 else k2_t
                        qm_t = q_t[h // 2]

                        def emit_scores(js, i=i, pr=pr, kh_t=kh_t, qm_t=qm_t):
                            offs = [max(0, jj - 4 * i) * 128 for jj in js]
                            pss = psS.tile([128, 2, 512], f32, tag="pss", name="pss")
                            for u in range(2):
                                nc.tensor.matmul(
                                    pss[:, u, offs[u]:512],
                                    kh_t[:, ts(js[u], 128)],
                                    qm_t[:, i * 512 + offs[u]:(i + 1) * 512],
                                    start=True, stop=True,
                                )
                            nc.scalar.activation(
                                pr[:, js[0]:js[0] + 2, offs[0]:512],
                                pss[:, :, offs[0]:512],
                                AF.Exp, scale=SCALE,
                            )
                            for u in range(2):
                                r = js[u] - 4 * i
                                if r >= 0:  # block overlapping the causal diagonal
                                    off = offs[u]
                                    nc.vector.tensor_tensor(
                                        pr[:, js[u], off:off + 128],
                                        pr[:, js[u], off:off + 128],
                                        tri_t, OP.mult,
                                    )

                        def emit_pv(js, i=i, pr=pr, po=po, nj=nj):
                            for jj in js:
                                off = max(0, jj - 4 * i) * 128
                                nc.tensor.matmul(
                                    po[:, off:512], vaug_t[:, jj, :], pr[:, jj, off:512],
                                    start=(jj == 0), stop=(jj == nj - 1),
                                )

                        LAG = 2
                        for ci, ch in enumerate(chunks):
                            emit_scores(ch)
                            if ci >= LAG:
                                emit_pv(chunks[ci - LAG])
                        for ci in range(max(0, len(chunks) - LAG), len(chunks)):
                            emit_pv(chunks[ci])
                        # normalize: oT[f, q] = po[f, q] / den[q]; den row broadcast
                        # across partitions via a K=1 ones matmul, then 1/x on DVE
                        dbc = btmp.tile([D + 1, 512], bf16, tag="dbc")
                        nc.vector.tensor_copy(dbc[D:D + 1, :], po[D:D + 1, :])
                        pb = psB.tile([D, 512], f32, tag="pb")
                        nc.tensor.matmul(
                            pb, ones_t[D:D + 1, 0:D], dbc[D:D + 1, :],
                            start=True, stop=True,
                        )
                        rbs = btmp.tile([D, 512], f32, tag="rbs")
                        nc.vector.reciprocal_approx_fast(out=rbs, in_=pb)
                        oth = btmp.tile([D, 512], bf16, tag="oth")
                        nc.vector.tensor_tensor(oth, po[0:D, :], rbs, OP.mult)
                        for half in range(2):
                            dd = nc.sync.dma_start(
                                a2a_in[h].ap()[2 * i + half, :, :],
                                oth[:, ts(half, 256)],
                            )
                            a2a_dmas.append((h, dd))
                    # head h's outputs are fully staged: fire its AllToAll now
                    # so it overlaps the next head's attention compute, then
                    # pull its o_proj chunks in on the gpsimd queue
                    ccs.append(_emit_cc(h))
                    _emit_lo(h)

        # ---- Phase D: o_proj for this core's 256 seq rows ----
        # k chunks 0..11 (heads 0-2) have their collectives done well before
        # attention drains; k 12..15 wait only on the last head's collective.
        # Run all 8 PSUM groups' first 12 chunks, then the last 4 per group.
        with nc.named_scope("oproj"):
            with (
                tc.tile_pool(name="dtmp", bufs=3) as dtmp,
                tc.tile_pool(name="psD", bufs=1, space="PSUM") as psD,
            ):
                groups = [(m, e4) for m in range(SPC // 128) for e4 in range(HID // 512)]
                psos = [
                    psD.tile([128, 512], f32, tag=f"pso{g}", name=f"pso{g}")
                    for g in range(len(groups))
                ]
                KA = 12
                for g, (m, e4) in enumerate(groups):
                    for k in range(KA):
                        nc.tensor.matmul(
                            psos[g],
                            lo_t[:, k, ts(m, 128)],
                            wo_t[:, k, ts(e4, 512)],
                            start=(k == 0),
                            stop=False,
                        )
                for g, (m, e4) in enumerate(groups):
                    for k in range(KA, KO):
                        nc.tensor.matmul(
                            psos[g],
                            lo_t[:, k, ts(m, 128)],
                            wo_t[:, k, ts(e4, 512)],
                            start=False,
                            stop=(k == KO - 1),
                        )
                    ob = dtmp.tile([128, 512], f32, tag="ob")
                    nc.vector.tensor_copy(ob, psos[g])
                    nc.sync.dma_start(out.ap()[ts(m, 128), ts(e4, 512)], ob)

    nc.compile()
    return nc


def _get_nc():
    if "nc" not in _CACHE:
        _CACHE["nc"] = _build()
    return _CACHE["nc"]


def _host_prep(hidden_states, positions, w_qkv, w_o):
    bf16 = ml_dtypes.bfloat16
    hTb = np.ascontiguousarray(hidden_states.astype(np.float32).T).astype(bf16)
    # pretile to [n, p, ko, s] so each 512-seq chunk is one contiguous DMA
    hTt = np.ascontiguousarray(
        hTb.reshape(HID // 128, 128, T // 512, 512).transpose(2, 1, 0, 3)
    )
    woTb = np.ascontiguousarray(w_o.astype(np.float32).T).astype(bf16)
    # o_proj contraction order matches the per-head AllToAll arrival order:
    # head h of every core, h = 0..3
    rows = np.concatenate(
        [
            (np.arange(NCORES)[:, None] * FPC + h * D + np.arange(D)[None, :]).reshape(-1)
            for h in range(HPC)
        ]
    )
    woTb = np.ascontiguousarray(woTb[rows])

    inv = 1.0 / (ROPE_THETA ** (np.arange(0, D, 2, dtype=np.float32) / D))  # [32]
    ang = positions.astype(np.float32)[:, None] * inv[None, :]              # [T, 32]
    cos = np.cos(ang).T  # [32, T]
    sin = np.sin(ang).T
    p = np.arange(128)
    fr = (p % D) % (D // 2)
    sgn = np.where((p % D) < (D // 2), -1.0, 1.0).astype(np.float32)
    cosf = np.ascontiguousarray(cos[fr]).astype(bf16)                 # [128, T]
    sinf = np.ascontiguousarray(sin[fr] * sgn[:, None]).astype(bf16)  # [128, T]

    partner = np.where((p % D) < (D // 2), p + D // 2, p - D // 2)
    perm = np.zeros((128, 128), dtype=np.float32)
    perm[p, partner] = 1.0
    ident = np.eye(128, dtype=np.float32)
    tri = (np.arange(128)[None, :] >= np.arange(128)[:, None]).astype(np.float32)
    ones_m = np.ones((128, 128), dtype=np.float32)

    q_size = NH * D
    kv_size = NKV * D
    in_maps = []
    for c in range(NCORES):
        wq = w_qkv[c * FPC:(c + 1) * FPC]
        wk = w_qkv[q_size + c * D:q_size + (c + 1) * D]
        wv = w_qkv[q_size + kv_size + c * D:q_size + kv_size + (c + 1) * D]
        wTc = np.ascontiguousarray(
            np.concatenate([wq, wk, wv], axis=0).astype(np.float32).T
        ).astype(bf16)
        in_maps.append(
            {
                "hT": hTt,
                "wT": wTc,
                "cosf": cosf,
                "sinf": sinf,
                "perm": perm.astype(bf16),
                "ident": ident.astype(bf16),
                "tri": tri.astype(bf16),
                "ones": ones_m.astype(bf16),
                "woT": woTb,
            }
        )
    return in_maps


def run(inputs, trace=False):
    """Run on 8 NeuronCores; returns (full_output, BassKernelResults)."""
    if trace:
        _ensure_trace_hooks()
    from concourse import bass_utils

    if trace:
        bass_utils.upload_artifacts = lambda tmpdir: tmpdir
    nc = _get_nc()
    in_maps = _host_prep(
        np.asarray(inputs["hidden_states"]),
        np.asarray(inputs["positions"]),
        np.asarray(inputs["w_qkv"]),
        np.asarray(inputs["w_o"]),
    )
    res = bass_utils.run_bass_kernel_spmd(
        nc, in_maps, core_ids=list(range(NCORES)), trace=trace
    )
    full = np.concatenate(
        [res.results[c]["out"] for c in range(NCORES)], axis=0
    ).astype(np.float32)
    return full, res


def kernel(**inputs) -> np.ndarray:
    trace = bool(os.environ.get("KERNEL_TRACE"))
    full, _ = run(inputs, trace=trace)
    return full

